# revision 20
# baseline (speedup 1.0000x reference)
"""Trainium2 Bass kernel for nn_Contraction (gnn_message_passing).

Pure data-parallel over edges E across 8 NeuronCores. Per core, edges are
processed in tiles of 256 (2 subgroups g of 128):
  - x/y loaded e-major (contiguous DMA), PE-transposed to channel-major
    layout [p=(c + 64*g), (component, e')], cast to bf16.
  - radial MLP (silu(radial@W1)@W2 * cutoff) on PE/ACT -> conv_w, c-major.
  - 23 tensor-product paths: elementwise products + m-contraction on DVE
    (heaviest path on GPSIMD); per-path conv_w scaling folded into the
    cheapest side; 4 low-order paths derived as traces of shared
    intermediates.
  - per-output-order linear layers as PE matmuls contracting channels,
    row-tiled so both g-halves run concurrently; accumulate over paths in
    PSUM; outputs stored c-major, de-transposed on host.
"""

import sys

sys.path.insert(0, "/opt/trn_rl_repo")

from contextlib import ExitStack

import numpy as np

E_FULL = 20000
E_PAD = 20480
NCORES = 8
EC = E_PAD // NCORES          # 2560 edges per core
TE = 256                      # edges per tile
NT = EC // TE                 # 10 tiles
EPG = 128                     # edges per g-subgroup
C = 64
NB = 8
NCOMB = 23
CW = NCOMB * C                # 1472
RBASE = [0, 1, 4, 13]
NCOMP = 40


def _make_combs():
    combs = []
    for r1 in range(4):
        for r2 in range(4):
            for ro in range(4):
                s = r1 + r2 - ro
                if s >= 0 and s % 2 == 0 and s // 2 <= min(r1, r2):
                    combs.append((r1, r2, ro))
    return combs


COMBS = _make_combs()
RO_PATHS = {ro: [i for i, cb in enumerate(COMBS) if cb[2] == ro] for ro in range(4)}
LD_CNT = {ro: len(RO_PATHS[ro]) for ro in range(4)}

# derived combs: traces of a shared partner intermediate G (unscaled):
# dst -> (src, base_offsets (3 terms), out_step, out_cnt)
DERIVED = {
    21: (22, [0, 4, 8], 1, 1),
    13: (14, [0, 4, 8], 1, 1),
    15: (16, [0, 10, 20], 3, 3),
    19: (20, [0, 4, 8], 9, 3),
}
SHARED_SRC = {22, 14, 16, 20}
GPSIMD_COMBS = {22}  # offload heaviest product sets to GPSIMD


def _fold_side(i):
    r1, r2, ro = COMBS[i]
    if i in SHARED_SRC:
        return "out"
    m = min(3 ** r1, 3 ** r2, 3 ** ro)
    if 3 ** r1 == m:
        return "x"
    if 3 ** r2 == m:
        return "y"
    return "out"


_BUILD_CACHE = {}


def _build():
    if "nc" in _BUILD_CACHE:
        return _BUILD_CACHE["nc"]
    import concourse.bacc as bacc
    import concourse.bass as bass
    import concourse.mybir as mybir
    from concourse.tile import TileContext
    from concourse.masks import make_identity

    f32 = mybir.dt.float32
    bf16 = mybir.dt.bfloat16
    AF = mybir.ActivationFunctionType
    ALU = mybir.AluOpType
    AP = bass.AP

    def mk(tile_or_ap, off, freedims, pcount=None):
        """AP on same tensor: partition entry copied from the arg (count
        overridable), free dims = [step,count] element pairs, off = element
        offset into the free space (plus base offset)."""
        base = tile_or_ap[:, :] if not isinstance(tile_or_ap, AP) else tile_or_ap
        p = list(base.ap[0])
        if pcount is not None:
            p = [p[0], pcount]
        return AP(base.tensor, base.offset + off, [p] + [list(d) for d in freedims])

    def pshift(apx, nparts):
        """shift base partition of an AP by nparts (for partition slicing)."""
        return apx  # partition slicing done via tile[a:b, ...] instead

    nc = bacc.Bacc(
        "TRN2",
        target_bir_lowering=False,
        debug=False,
        enable_asserts=False,
        num_devices=NCORES,
    )

    xs = [nc.dram_tensor(f"x{r}", [EC, C * 3 ** r], f32, kind="ExternalInput")
          for r in range(4)]
    ys = [nc.dram_tensor(f"y{r}", [EC, C * 3 ** r], f32, kind="ExternalInput")
          for r in range(4)]
    radial = nc.dram_tensor("radial", [EC, NB], f32, kind="ExternalInput")
    cutoff = nc.dram_tensor("cutoff", [128, EC], f32, kind="ExternalInput")
    W1 = nc.dram_tensor("W1", [NB, C], f32, kind="ExternalInput")
    W2 = nc.dram_tensor("W2", [C, CW], f32, kind="ExternalInput")
    lds = [nc.dram_tensor(f"ld{ro}", [LD_CNT[ro] * C, C], f32, kind="ExternalInput")
           for ro in range(4)]
    outs = [nc.dram_tensor(f"o{ro}", [C, (3 ** ro) * EC], f32, kind="ExternalOutput")
            for ro in range(4)]

    with TileContext(nc) as tc, ExitStack() as ctx:
        cpool = ctx.enter_context(tc.tile_pool(name="const", bufs=1))
        state = ctx.enter_context(tc.tile_pool(name="state", bufs=1))
        xepool = ctx.enter_context(tc.tile_pool(name="xe", bufs=2))
        xcpool = ctx.enter_context(tc.tile_pool(name="xc", bufs=2))
        cwpool = ctx.enter_context(tc.tile_pool(name="cw", bufs=2))
        wpool = ctx.enter_context(tc.tile_pool(name="w", bufs=4))
        tpool = ctx.enter_context(tc.tile_pool(name="T", bufs=1))
        spool = ctx.enter_context(tc.tile_pool(name="scr", bufs=1))
        rpool = ctx.enter_context(tc.tile_pool(name="rad", bufs=2))
        trps = ctx.enter_context(tc.tile_pool(name="trps", bufs=1, space="PSUM"))
        cwps = ctx.enter_context(tc.tile_pool(name="cwps", bufs=2, space="PSUM"))
        ldps = ctx.enter_context(tc.tile_pool(name="ldps", bufs=2, space="PSUM"))

        ident = cpool.tile([128, 128], bf16)
        make_identity(nc, ident)
        W1sb = cpool.tile([NB, C], f32)
        nc.sync.dma_start(out=W1sb[:, :], in_=W1[:, :])
        W2sb = cpool.tile([C, CW], f32)
        nc.sync.dma_start(out=W2sb[:, :], in_=W2[:, :])
        ldsb = []
        for ro in range(4):
            cnt = LD_CNT[ro]
            lt = cpool.tile([128, cnt * C], bf16, tag=f"ld{ro}")
            src = AP(lds[ro][:, :].tensor, 0, [[C, C], [C * C, cnt], [1, C]])
            nc.gpsimd.dma_start(out=lt[0:64, :], in_=src)
            nc.gpsimd.dma_start(out=lt[64:128, :], in_=src)
            ldsb.append(lt)

        # h^T = silu(W1^T @ radial^T), resident [64, EC] f32
        hT = state.tile([C, EC], f32)
        for ch in range(EC // 512):
            rch = rpool.tile([NB, 512], f32, tag="rad")
            nc.sync.dma_start(
                out=rch[:, :],
                in_=AP(radial[:, :].tensor, ch * 512 * NB, [[1, NB], [NB, 512]]),
            )
            psh = ldps.tile([C, 512], f32, tag="lp0")
            nc.tensor.matmul(psh[:, :], W1sb[:, :], rch[:, :],
                             start=True, stop=True)
            sg = rpool.tile([C, 512], f32, tag="sg")
            nc.scalar.activation(sg[:, :], psh[:, :], AF.Sigmoid)
            nc.vector.tensor_mul(out=hT[:, ch * 512:(ch + 1) * 512],
                                 in0=psh[:, :], in1=sg[:, :])

        comp_list = []
        for r in range(4):
            comp_list += [(r, cc) for cc in range(3 ** r)]

        for t in range(NT):
            # ---- e-major staged loads ----
            xe, ye = [], []
            for r in range(4):
                tr = 3 ** r
                xt = xepool.tile([EPG, 2 * C * tr], bf16, tag=f"xe{r}")
                yt = xepool.tile([EPG, 2 * C * tr], bf16, tag=f"ye{r}")
                for tile_, dram in ((xt, xs[r]), (yt, ys[r])):
                    nc.gpsimd.dma_start(
                        out=tile_[:, :],
                        in_=AP(dram[:, :].tensor, t * TE * C * tr,
                               [[C * tr, EPG], [EPG * C * tr, 2], [1, C * tr]]),
                    )
                xe.append(xt)
                ye.append(yt)

            # ---- conv_w for this tile ----
            cut = rpool.tile([128, TE], f32, tag="cut")
            nc.sync.dma_start(
                out=cut[:, :],
                in_=AP(cutoff[:, :].tensor, t * TE, [[EC, 128], [1, TE]]))
            cwt = []
            for chk in range(12):
                csz = 128 if chk < 11 else 64
                pw = cwps.tile([128, TE], f32, tag="cwps")
                nc.tensor.matmul(
                    pw[:csz, :], W2sb[:, chk * 128: chk * 128 + csz],
                    hT[:, t * TE:(t + 1) * TE], start=True, stop=True,
                )
                ct = cwpool.tile([128, TE], bf16, tag=f"cw{chk}")
                nc.vector.scalar_tensor_tensor(
                    out=ct[:csz, :], in0=pw[:csz, :], scalar=1.0,
                    in1=cut[:csz, :],
                    op0=ALU.mult, op1=ALU.mult,
                )
                cwt.append(ct)

            # ---- transpose x/y to channel-major bf16 [128(c,g), (comp, e')] ----
            Xc = xcpool.tile([128, NCOMP * EPG], bf16, tag="Xc")
            Yc = xcpool.tile([128, NCOMP * EPG], bf16, tag="Yc")
            for dst, src_tiles in ((Xc, xe), (Yc, ye)):
                for g0 in range(0, NCOMP, 16):
                    gsz = min(16, NCOMP - g0)
                    pt = trps.tile([128, 16 * EPG], bf16, tag="trps")
                    for j in range(gsz):
                        r, cc = comp_list[g0 + j]
                        tr = 3 ** r
                        st = src_tiles[r]
                        in_ap = mk(st, cc, [[C * tr, 2], [tr, C]])
                        nc.tensor.transpose(
                            pt[:, j * EPG:(j + 1) * EPG], in_ap, ident[:, :])
                    nc.scalar.copy(
                        out=dst[:, g0 * EPG:(g0 + gsz) * EPG],
                        in_=pt[:, :gsz * EPG])

            # ---- per-comb products, phased by ro (3->0) with slot reuse ----
            T = {}
            wtiles = {}

            def load_w(i):
                wt = wpool.tile([128, EPG], bf16, tag="w")
                chk, half = i // 2, i % 2
                for g in range(2):
                    nc.sync.dma_start(
                        out=wt[g * 64:(g + 1) * 64, :],
                        in_=cwt[chk][half * 64:half * 64 + 64,
                                     g * EPG:(g + 1) * EPG],
                    )
                return wt

            def wbc(wt, n, pc=128, poff=0):
                b = wt[:, :] if poff == 0 else wt[poff:poff + pc, :]
                return AP(b.tensor, b.offset,
                          [[b.ap[0][0], pc], [0, n], [1, EPG]])

            PHASE_TAGS = {}  # comb -> (tag, size_units)

            def emit_products(i, eng_override=None):
                r1, r2, ro = COMBS[i]
                k = (r1 + r2 - ro) // 2
                A, B, M = 3 ** (r1 - k), 3 ** (r2 - k), 3 ** k
                S = A * B
                eng = nc.gpsimd if i in GPSIMD_COMBS else nc.vector
                fs = _fold_side(i)
                wt = load_w(i)
                wtiles[i] = wt

                xsrc, xoff = Xc, RBASE[r1] * EPG
                ysrc, yoff = Yc, RBASE[r2] * EPG
                if fs == "x":
                    xw = spool.tile([128, 9 * EPG], bf16, tag="fw")
                    eng.tensor_mul(
                        out=mk(xw, 0, [[1, 3 ** r1 * EPG]]),
                        in0=mk(Xc, xoff, [[1, 3 ** r1 * EPG]]),
                        in1=wbc(wt, 3 ** r1))
                    xsrc, xoff = xw, 0
                elif fs == "y":
                    yw = spool.tile([128, 9 * EPG], bf16, tag="fw")
                    eng.tensor_mul(
                        out=mk(yw, 0, [[1, 3 ** r2 * EPG]]),
                        in0=mk(Yc, yoff, [[1, 3 ** r2 * EPG]]),
                        in1=wbc(wt, 3 ** r2))
                    ysrc, yoff = yw, 0

                tag, sz = PHASE_TAGS[i]
                Ti = tpool.tile([128, sz * EPG], bf16, tag=tag)
                T[i] = Ti
                pp = spool.tile([128, 27 * EPG], bf16, tag="pp")
                for m in range(M):
                    xap = mk(xsrc, xoff + m * EPG,
                             [[M * EPG, A], [0, B], [1, EPG]])
                    yap = mk(ysrc, yoff + m * EPG,
                             [[0, A], [M * EPG, B], [1, EPG]])
                    dstt = Ti if m == 0 else pp
                    eng.tensor_mul(
                        out=mk(dstt, 0, [[B * EPG, A], [EPG, B], [1, EPG]]),
                        in0=xap, in1=yap)
                    if m > 0:
                        eng.tensor_add(
                            out=mk(Ti, 0, [[1, S * EPG]]),
                            in0=mk(Ti, 0, [[1, S * EPG]]),
                            in1=mk(pp, 0, [[1, S * EPG]]))
                if fs == "out" and i not in SHARED_SRC:
                    eng.tensor_mul(
                        out=mk(Ti, 0, [[1, S * EPG]]),
                        in0=mk(Ti, 0, [[1, S * EPG]]),
                        in1=wbc(wt, S))

            def emit_derived(i):
                src_i, offs, ostep, ocnt = DERIVED[i]
                ro = COMBS[i][2]
                S = 3 ** ro
                G = T[src_i]
                wt = load_w(i)
                tag, sz = PHASE_TAGS[i]
                Ti = tpool.tile([128, sz * EPG], bf16, tag=tag)
                T[i] = Ti

                def gap(aoff):
                    return mk(G, aoff * EPG, [[ostep * EPG, ocnt], [1, EPG]])

                tap = mk(Ti, 0, [[EPG, ocnt], [1, EPG]])
                nc.vector.tensor_add(out=tap, in0=gap(offs[0]), in1=gap(offs[1]))
                nc.vector.tensor_add(out=tap, in0=tap, in1=gap(offs[2]))
                nc.vector.tensor_mul(out=tap, in0=tap, in1=wbc(wt, S))

            def emit_fold_shared(i):
                S = 3 ** COMBS[i][2]
                nc.vector.tensor_mul(
                    out=mk(T[i], 0, [[1, S * EPG]]),
                    in0=mk(T[i], 0, [[1, S * EPG]]),
                    in1=wbc(wtiles[i], S))

            def emit_ld(ro):
                S = 3 ** ro
                paths = RO_PATHS[ro]
                s0 = 0
                while s0 < S:
                    scnt = min(4, S - s0)
                    pg0 = ldps.tile([C, 512], f32, tag="lp0")
                    pg1 = ldps.tile([C, 512], f32, tag="lp1")
                    pgs = [pg0, pg1]
                    for j, i in enumerate(paths):
                        for g in range(2):
                            rhs_base = T[i][g * 64:(g + 1) * 64, :]
                            rhs = AP(rhs_base.tensor,
                                     rhs_base.offset + s0 * EPG,
                                     [list(rhs_base.ap[0])] +
                                     [[1, scnt * EPG]])
                            lhs_base = ldsb[ro][g * 64:(g + 1) * 64, :]
                            lhs = AP(lhs_base.tensor, lhs_base.offset + j * C,
                                     [list(lhs_base.ap[0])] + [[1, C]])
                            nc.tensor.matmul(
                                pgs[g][:, :scnt * EPG], lhs, rhs,
                                start=(j == 0), stop=(j == len(paths) - 1),
                            )
                    for g in range(2):
                        ost = rpool.tile([C, 512], f32, tag="ost")
                        nc.scalar.copy(out=ost[:, :scnt * EPG],
                                       in_=pgs[g][:, :scnt * EPG])
                        nc.sync.dma_start(
                            out=AP(outs[ro][:, :].tensor,
                                   s0 * EC + t * TE + g * EPG,
                                   [[S * EC, C], [EC, scnt], [1, EPG]]),
                            in_=ost[:, :scnt * EPG],
                        )
                    s0 += scnt

            # slot assignment: big phase slots P0..P6 reused ro3 -> ro2 -> ro1
            # -> ro0; derived combs get small persistent slots D*.
            for ph_ro in (3, 2, 1, 0):
                live = [i for i in RO_PATHS[ph_ro] if i not in DERIVED]
                for slot, i in enumerate(live):
                    PHASE_TAGS[i] = (f"P{slot}", 27 if ph_ro >= 3 else
                                     (9 if ph_ro == 2 else 3 ** ph_ro))
            for i in DERIVED:
                PHASE_TAGS[i] = (f"D{i}", 3 ** COMBS[i][2])

            # ro=3 phase (includes shared srcs 16, 20 -> derive 15, 19 now)
            for i in RO_PATHS[3]:
                emit_products(i)
            emit_derived(15)
            emit_derived(19)
            emit_fold_shared(16)
            emit_fold_shared(20)
            emit_ld(3)
            # ro=2 phase (shared srcs 22, 14 -> derive 21, 13 now)
            for i in RO_PATHS[2]:
                emit_products(i)
            emit_derived(21)
            emit_derived(13)
            emit_fold_shared(22)
            emit_fold_shared(14)
            emit_ld(2)
            # ro=1 phase
            for i in RO_PATHS[1]:
                if i not in DERIVED:
                    emit_products(i)
            emit_ld(1)
            # ro=0 phase
            for i in RO_PATHS[0]:
                if i not in DERIVED:
                    emit_products(i)
            emit_ld(0)

    nc.compile()
    _BUILD_CACHE["nc"] = nc
    return nc


def kernel(**inputs):
    nc = _build()
    from concourse.bass_utils import run_bass_kernel_spmd

    pad = E_PAD - E_FULL

    def prep(a):
        a = np.ascontiguousarray(np.asarray(a, dtype=np.float32))
        a = a.reshape(a.shape[0], -1)
        if pad:
            a = np.concatenate([a, np.zeros((pad, a.shape[1]), np.float32)], 0)
        return a

    full = {}
    for r in range(4):
        full[f"x{r}"] = prep(inputs[f"x{r}"])
        full[f"y{r}"] = prep(inputs[f"y{r}"])
    full["radial"] = prep(inputs["radial"])
    cutflat = prep(inputs["cutoff"])

    in_maps = []
    for core in range(NCORES):
        sl = slice(core * EC, (core + 1) * EC)
        m = {k: np.ascontiguousarray(v[sl]) for k, v in full.items()}
        m["cutoff"] = np.ascontiguousarray(
            np.broadcast_to(cutflat[sl, 0][None, :], (128, EC)))
        m["W1"] = np.ascontiguousarray(np.asarray(inputs["W1"], np.float32))
        m["W2"] = np.ascontiguousarray(np.asarray(inputs["W2"], np.float32))
        for ro in range(4):
            m[f"ld{ro}"] = np.ascontiguousarray(
                np.asarray(inputs[f"ld{ro}"], np.float32))
        in_maps.append(m)

    import time as _time
    _t0 = _time.time()
    res = run_bass_kernel_spmd(nc, in_maps, core_ids=list(range(NCORES)))
    _dt = _time.time() - _t0
    print(f"spmd call wall: {_dt*1e3:.1f} ms")
    out = []
    for ro in range(4):
        S = 3 ** ro
        parts = [res.results[c][f"o{ro}"].reshape(C, S, EC)
                 for c in range(NCORES)]
        fullo = np.concatenate(parts, axis=2).transpose(2, 0, 1)[:E_FULL]
        if ro == 0:
            out.append(np.ascontiguousarray(fullo[:, :, 0]))
        else:
            out.append(np.ascontiguousarray(
                fullo.reshape((E_FULL, C) + (3,) * ro)))
    return tuple(out)
